# revision 7
# baseline (speedup 1.0000x reference)
"""MultiHuberLoss Trainium2 kernel (bf16 stream, multi-engine reduction).

Reference (per element, with m = +x at the target class, -x elsewhere):
    hinge = max(0, 1 - m);  loss = where(m >= -1, hinge^2, -4m);  out = sum(loss)/N

Math (exact identities), treating every element as non-target (m = -x):
    G(x) = (v+1)^2 + 4*u - 4,  v = clamp(x,-1,1), u = max(x,1)
Per-row correction for the target column t:  -4*x_t.

Host-side prep (layout/precision only): cast to bf16; swap each row's
target element into column 0 (per-row loss is permutation invariant),
so the correction reads a strided slice instead of a gather.

Device per core (8192 rows = [128 partitions, 64000 free] bf16), 8 tiles
of [128, 8000].  Fused-accumulation ops all run at 1x, so the two
reductions (sum of squares, sum of u) are spread across every engine:
  - DVE (4x tensor_scalar): v = clamp(x,-1,1); u4 = 4*max(x,1) for the
    PE-summed columns [0:PB)
  - ACT: Square(v+1) + accum on columns [0:NA)   (1x, the main reducer)
  - DVE STT (v+2)*v + accum on [NA:FD)           (1x on leftover)
  - PE:  ones^T @ u4 in FD=500 chunks accumulated into one PSUM bank
  - DVE TS max+reduce-add on [PB:FD) for the u leftover (1x)
  - GPSIMD: u4 prep on alternating tiles (relieves DVE)
  - correction: -4 * x[:, j*1000] strided, DVE accum (tiny)
"""

import ml_dtypes
import numpy as np

import concourse.bacc as bacc
import concourse.mybir as mybir
from concourse.bass_utils import run_bass_kernel_spmd
from concourse.tile import TileContext

N_TOTAL = 65536
C = 1000
N_CORES = 8
ROWS = N_TOTAL // N_CORES  # 8192 rows per core
P = 128                    # partitions
JPP = ROWS // P            # 64 rows per partition
FREE = JPP * C             # 64000 bf16 per partition

TILE_FDS = [8000] * 8
assert sum(TILE_FDS) == FREE
NA = 7000        # ACT-square columns per tile (rest: DVE STT)
PB = 6500        # PE-summed u columns per tile (rest: DVE TS reduce)
CHUNK = 500      # PE matmul moving free dim
GP_U_TILES = (1, 3, 5, 7)   # tiles whose u4-prep runs on GPSIMD

f32 = mybir.dt.float32
bf16 = mybir.dt.bfloat16
Alu = mybir.AluOpType
AF = mybir.ActivationFunctionType

NT = len(TILE_FDS)


def build_program():
    nc = bacc.Bacc(
        "TRN2", target_bir_lowering=False, debug=False, num_devices=N_CORES
    )
    x = nc.dram_tensor("x", [ROWS, C], bf16, kind="ExternalInput")
    out = nc.dram_tensor("out", [1, 1], f32, kind="ExternalOutput")

    x_flat = x.ap().rearrange("(p j) c -> p (j c)", p=P)  # [128, 64000]

    n_stt_pp = sum(fd - NA for fd in TILE_FDS)
    # per-partition: +count for the STT region, -4 per element
    bias_c = (P * (n_stt_pp - 4.0 * FREE)) / N_TOTAL

    n_chunks_total = NT * (PB // CHUNK)

    with TileContext(nc) as tc:
        with (
            tc.tile_pool(name="xp", bufs=4) as xp,
            tc.tile_pool(name="vp", bufs=2) as vp,
            tc.tile_pool(name="up", bufs=3) as up,
            tc.tile_pool(name="scr", bufs=1) as scr,
            tc.tile_pool(name="small", bufs=1) as small,
            tc.tile_pool(name="psp", bufs=1, space="PSUM") as psp,
        ):
            max_fd = max(TILE_FDS)
            sq_scr = scr.tile([P, NA], bf16, tag="sq_scr")
            stt_scr = scr.tile([P, max_fd - NA], bf16, tag="stt_scr")
            ub_scr = scr.tile([P, max_fd - PB], bf16, tag="ub_scr")
            c0_scr = scr.tile([P, 8], f32, tag="c0_scr")
            # acc cols: [0:NT) u-leftover sums (x4 later), [NT:2NT) ACT sq,
            # [2NT:3NT) STT sq, [3NT:4NT) col0 correction
            acc = small.tile([P, 4 * NT], f32, tag="acc")
            nc.vector.memset(acc[:], 0.0)
            ones_bf = small.tile([P, 1], bf16, tag="ones_bf")
            nc.vector.memset(ones_bf[:], 1.0)
            ones_f = small.tile([P, 1], f32, tag="ones_f")
            nc.vector.memset(ones_f[:], 1.0)
            psB = psp.tile([1, CHUNK], f32, tag="psB")

            ci = 0
            off = 0
            for t, fd in enumerate(TILE_FDS):
                xt = xp.tile([P, fd], bf16)
                nc.sync.dma_start(out=xt[:], in_=x_flat[:, off:off + fd])
                v = vp.tile([P, fd], bf16)
                nc.vector.tensor_scalar(
                    v[:], xt[:], -1.0, 1.0, Alu.max, Alu.min
                )
                nc.scalar.activation(
                    sq_scr[:, 0:NA], v[:, 0:NA], AF.Square,
                    bias=1.0, scale=1.0,
                    accum_out=acc[:, NT + t:NT + t + 1],
                )
                # u4 = 4*max(x,1) on [0:PB) -> PE column sums
                u4 = up.tile([P, PB], bf16)
                eng = nc.gpsimd if t in GP_U_TILES else nc.vector
                eng.tensor_scalar(
                    u4[:], xt[:, 0:PB], 1.0, 4.0, Alu.max, Alu.mult
                )
                for c in range(PB // CHUNK):
                    nc.tensor.matmul(
                        out=psB[:, 0:CHUNK],
                        lhsT=ones_bf[:],
                        rhs=u4[:, c * CHUNK:(c + 1) * CHUNK],
                        start=(ci == 0),
                        stop=(ci == n_chunks_total - 1),
                    )
                    ci += 1
                # u leftover [PB:fd): max + reduce-add (1x)
                nc.vector.tensor_scalar(
                    ub_scr[:, 0:fd - PB], xt[:, PB:fd], 1.0, 0.0,
                    Alu.max, Alu.add,
                    accum_out=acc[:, t:t + 1],
                )
                # STT square leftover [NA:fd)
                nc.vector.scalar_tensor_tensor(
                    out=stt_scr[:, 0:fd - NA],
                    in0=v[:, NA:fd], scalar=2.0, in1=v[:, NA:fd],
                    op0=Alu.add, op1=Alu.mult,
                    accum_out=acc[:, 2 * NT + t:2 * NT + t + 1],
                )
                # correction: -4 * x[:, j*C]
                ncol = fd // C
                x3 = xt[:].rearrange("p (j c) -> p j c", c=C)
                nc.vector.tensor_scalar(
                    c0_scr[:, 0:ncol],
                    x3[:, :, 0:1].squeeze(2),
                    -4.0, 0.0, Alu.mult, Alu.add,
                    accum_out=acc[:, 3 * NT + t:3 * NT + t + 1],
                )
                off += fd
            assert ci == n_chunks_total

            # ---- final combine ----
            # u-leftover columns carry sum(u); B term needs 4*sum(u)
            nc.vector.tensor_scalar(
                acc[:, 0:NT], acc[:, 0:NT], 4.0, None, Alu.mult
            )
            s_p = small.tile([P, 1], f32, tag="s_p")
            nc.vector.reduce_sum(s_p, acc[:], axis=mybir.AxisListType.X)
            psS = psp.tile([1, 8], f32, tag="psS")
            nc.tensor.matmul(
                out=psS[:, 0:1], lhsT=ones_f[:], rhs=s_p[:],
                start=True, stop=True,
            )
            # sB = sum over the accumulated PE bank
            sb_scr = small.tile([1, CHUNK], f32, tag="sb_scr")
            sB = small.tile([1, 1], f32, tag="sB")
            nc.scalar.activation(
                sb_scr[:], psB[:, 0:CHUNK], AF.Identity,
                bias=0.0, scale=1.0, accum_out=sB[:],
            )
            tmp = small.tile([1, 1], f32, tag="tmp")
            nc.vector.scalar_tensor_tensor(
                out=tmp[:], in0=sB[:], scalar=1.0, in1=psS[:, 0:1],
                op0=Alu.mult, op1=Alu.add,
            )
            bias_t = small.tile([1, 1], f32, tag="bias")
            nc.vector.memset(bias_t[:], bias_c)
            res = small.tile([1, 1], f32, tag="res")
            nc.scalar.activation(
                res[:], tmp[:], AF.Identity,
                bias=bias_t[:], scale=1.0 / N_TOTAL,
            )
            nc.sync.dma_start(out=out.ap(), in_=res[:])

    nc.compile()
    return nc


_NC_CACHE = None
LAST_RESULTS = None


def kernel(input, target):
    global _NC_CACHE, LAST_RESULTS
    x = np.asarray(input)
    tg = np.asarray(target).astype(np.int64)
    assert x.shape == (N_TOTAL, C), x.shape
    assert tg.shape == (N_TOTAL,), tg.shape

    if _NC_CACHE is None:
        _NC_CACHE = build_program()
    nc = _NC_CACHE

    xb = x.astype(ml_dtypes.bfloat16)
    rows = np.arange(N_TOTAL)
    tv = xb[rows, tg].copy()
    xb[rows, tg] = xb[rows, 0]
    xb[rows, 0] = tv

    in_maps = [
        {"x": xb[c * ROWS:(c + 1) * ROWS]}
        for c in range(N_CORES)
    ]
    res = run_bass_kernel_spmd(nc, in_maps, core_ids=list(range(N_CORES)))
    LAST_RESULTS = res
    total = np.float32(0.0)
    for r in res.results:
        total += np.float32(r["out"].reshape(()))
    return np.asarray(total, dtype=np.float32)


if __name__ == "__main__":
    rng = np.random.default_rng(0)
    xs = rng.standard_normal((N_TOTAL, C), dtype=np.float32)
    ts = rng.integers(0, C, size=(N_TOTAL,)).astype(np.int64)
    got = kernel(xs, ts)
    m = np.where(np.arange(C)[None, :] == ts[:, None], xs, -xs)
    hinge = np.maximum(0.0, 1.0 - m)
    loss = np.where(m >= -1.0, hinge * hinge, -4.0 * m)
    want = loss.sum(dtype=np.float64) / N_TOTAL
    print("got", got, "want", want, "rel", abs(got - want) / abs(want))


# revision 8
# speedup vs baseline: 7.1088x; 7.1088x over previous
"""MultiHuberLoss Trainium2 kernel (bf16 stream, multi-engine reduction).

Reference (per element, with m = +x at the target class, -x elsewhere):
    hinge = max(0, 1 - m);  loss = where(m >= -1, hinge^2, -4m);  out = sum(loss)/N

Math (exact identities), treating every element as non-target (m = -x):
    G(x) = (v+1)^2 + 4*u - 4,  v = clamp(x,-1,1), u = max(x,1)
Per-row correction for the target column t:  -4*x_t.

Host-side prep (layout/precision only): cast to bf16; swap each row's
target element into column 0 (per-row loss is permutation invariant),
so the correction reads a strided slice instead of a gather.

Device per core (8192 rows = [128 partitions, 64000 free] bf16), 8 tiles
of [128, 8000].  Fused-accumulation ops all run at 1x, so the two
reductions (sum of squares, sum of u) are spread across every engine:
  - DVE (4x tensor_scalar): v = clamp(x,-1,1); u4 = 4*max(x,1) for the
    PE-summed columns [0:PB)
  - ACT: Square(v+1) + accum on columns [0:NA)   (1x, the main reducer)
  - DVE STT (v+2)*v + accum on [NA:FD)           (1x on leftover)
  - PE:  ones^T @ u4 in FD=500 chunks accumulated into one PSUM bank
  - DVE TS max+reduce-add on [PB:FD) for the u leftover (1x)
  - GPSIMD: u4 prep on alternating tiles (relieves DVE)
  - correction: -4 * x[:, j*1000] strided, DVE accum (tiny)
"""

import ml_dtypes
import numpy as np

import concourse.bacc as bacc
import concourse.mybir as mybir
from concourse.bass_utils import run_bass_kernel_spmd
from concourse.tile import TileContext

N_TOTAL = 65536
C = 1000
N_CORES = 8
ROWS = N_TOTAL // N_CORES  # 8192 rows per core
P = 128                    # partitions
JPP = ROWS // P            # 64 rows per partition
FREE = JPP * C             # 64000 bf16 per partition

TILE_FDS = [8000] * 8
assert sum(TILE_FDS) == FREE
NA = 6900        # ACT-square columns per tile (rest: DVE STT)
PB = 8000        # PE-summed u columns per tile (rest: DVE TS reduce)
CHUNK = 500      # PE matmul moving free dim
GP_U_TILES = ()   # tiles whose u4-prep runs on GPSIMD

f32 = mybir.dt.float32
bf16 = mybir.dt.bfloat16
Alu = mybir.AluOpType
AF = mybir.ActivationFunctionType

NT = len(TILE_FDS)


def build_program():
    nc = bacc.Bacc(
        "TRN2", target_bir_lowering=False, debug=False, num_devices=N_CORES
    )
    x = nc.dram_tensor("x", [ROWS, C], bf16, kind="ExternalInput")
    out = nc.dram_tensor("out", [1, 1], f32, kind="ExternalOutput")

    x_flat = x.ap().rearrange("(p j) c -> p (j c)", p=P)  # [128, 64000]

    n_stt_pp = sum(fd - NA for fd in TILE_FDS)
    # per-partition: +count for the STT region, -4 per element
    bias_c = (P * (n_stt_pp - 4.0 * FREE)) / N_TOTAL

    n_chunks_total = NT * (PB // CHUNK)

    with TileContext(nc) as tc:
        with (
            tc.tile_pool(name="xp", bufs=4) as xp,
            tc.tile_pool(name="vp", bufs=2) as vp,
            tc.tile_pool(name="up", bufs=3) as up,
            tc.tile_pool(name="scr", bufs=1) as scr,
            tc.tile_pool(name="small", bufs=1) as small,
            tc.tile_pool(name="psp", bufs=1, space="PSUM") as psp,
        ):
            max_fd = max(TILE_FDS)
            sq_scr = scr.tile([P, NA], bf16, tag="sq_scr")
            stt_scr = scr.tile([P, max_fd - NA], bf16, tag="stt_scr")
            c0_scr = scr.tile([P, 8], f32, tag="c0_scr")
            # acc cols: [0:NT) u-leftover sums (x4 later), [NT:2NT) ACT sq,
            # [2NT:3NT) STT sq, [3NT:4NT) col0 correction
            acc = small.tile([P, 4 * NT], f32, tag="acc")
            nc.vector.memset(acc[:], 0.0)
            ones_bf = small.tile([P, 1], bf16, tag="ones_bf")
            nc.vector.memset(ones_bf[:], 1.0)
            ones_f = small.tile([P, 1], f32, tag="ones_f")
            nc.vector.memset(ones_f[:], 1.0)
            psB = psp.tile([1, CHUNK], f32, tag="psB")

            ci = 0
            off = 0
            for t, fd in enumerate(TILE_FDS):
                xt = xp.tile([P, fd], bf16)
                nc.sync.dma_start(out=xt[:], in_=x_flat[:, off:off + fd])
                v = vp.tile([P, fd], bf16)
                nc.vector.tensor_scalar(
                    v[:], xt[:], -1.0, 1.0, Alu.max, Alu.min
                )
                nc.scalar.activation(
                    sq_scr[:, 0:NA], v[:, 0:NA], AF.Square,
                    bias=1.0, scale=1.0,
                    accum_out=acc[:, NT + t:NT + t + 1],
                )
                # u4 = 4*max(x,1) on [0:PB) -> PE column sums
                u4 = up.tile([P, PB], bf16)
                eng = nc.gpsimd if t in GP_U_TILES else nc.vector
                eng.tensor_scalar(
                    u4[:], xt[:, 0:PB], 1.0, 4.0, Alu.max, Alu.mult
                )
                for c in range(PB // CHUNK):
                    nc.tensor.matmul(
                        out=psB[:, 0:CHUNK],
                        lhsT=ones_bf[:],
                        rhs=u4[:, c * CHUNK:(c + 1) * CHUNK],
                        start=(ci == 0),
                        stop=(ci == n_chunks_total - 1),
                    )
                    ci += 1
                # STT square leftover [NA:fd)
                nc.vector.scalar_tensor_tensor(
                    out=stt_scr[:, 0:fd - NA],
                    in0=v[:, NA:fd], scalar=2.0, in1=v[:, NA:fd],
                    op0=Alu.add, op1=Alu.mult,
                    accum_out=acc[:, 2 * NT + t:2 * NT + t + 1],
                )
                # correction: -4 * x[:, j*C]
                ncol = fd // C
                x3 = xt[:].rearrange("p (j c) -> p j c", c=C)
                nc.vector.tensor_scalar(
                    c0_scr[:, 0:ncol],
                    x3[:, :, 0:1].squeeze(2),
                    -4.0, 0.0, Alu.mult, Alu.add,
                    accum_out=acc[:, 3 * NT + t:3 * NT + t + 1],
                )
                off += fd
            assert ci == n_chunks_total

            # ---- final combine ----
            # u-leftover columns carry sum(u); B term needs 4*sum(u)
            nc.vector.tensor_scalar(
                acc[:, 0:NT], acc[:, 0:NT], 4.0, None, Alu.mult
            )
            s_p = small.tile([P, 1], f32, tag="s_p")
            nc.vector.reduce_sum(s_p, acc[:], axis=mybir.AxisListType.X)
            psS = psp.tile([1, 8], f32, tag="psS")
            nc.tensor.matmul(
                out=psS[:, 0:1], lhsT=ones_f[:], rhs=s_p[:],
                start=True, stop=True,
            )
            # sB = sum over the accumulated PE bank
            sb_scr = small.tile([1, CHUNK], f32, tag="sb_scr")
            sB = small.tile([1, 1], f32, tag="sB")
            nc.scalar.activation(
                sb_scr[:], psB[:, 0:CHUNK], AF.Identity,
                bias=0.0, scale=1.0, accum_out=sB[:],
            )
            tmp = small.tile([1, 1], f32, tag="tmp")
            nc.vector.scalar_tensor_tensor(
                out=tmp[:], in0=sB[:], scalar=1.0, in1=psS[:, 0:1],
                op0=Alu.mult, op1=Alu.add,
            )
            bias_t = small.tile([1, 1], f32, tag="bias")
            nc.vector.memset(bias_t[:], bias_c)
            res = small.tile([1, 1], f32, tag="res")
            nc.scalar.activation(
                res[:], tmp[:], AF.Identity,
                bias=bias_t[:], scale=1.0 / N_TOTAL,
            )
            nc.sync.dma_start(out=out.ap(), in_=res[:])

    nc.compile()
    return nc


_NC_CACHE = None
LAST_RESULTS = None


def kernel(input, target):
    global _NC_CACHE, LAST_RESULTS
    x = np.asarray(input)
    tg = np.asarray(target).astype(np.int64)
    assert x.shape == (N_TOTAL, C), x.shape
    assert tg.shape == (N_TOTAL,), tg.shape

    if _NC_CACHE is None:
        _NC_CACHE = build_program()
    nc = _NC_CACHE

    xb = x.astype(ml_dtypes.bfloat16)
    rows = np.arange(N_TOTAL)
    tv = xb[rows, tg].copy()
    xb[rows, tg] = xb[rows, 0]
    xb[rows, 0] = tv

    in_maps = [
        {"x": xb[c * ROWS:(c + 1) * ROWS]}
        for c in range(N_CORES)
    ]
    res = run_bass_kernel_spmd(nc, in_maps, core_ids=list(range(N_CORES)))
    LAST_RESULTS = res
    total = np.float32(0.0)
    for r in res.results:
        total += np.float32(r["out"].reshape(()))
    return np.asarray(total, dtype=np.float32)


if __name__ == "__main__":
    rng = np.random.default_rng(0)
    xs = rng.standard_normal((N_TOTAL, C), dtype=np.float32)
    ts = rng.integers(0, C, size=(N_TOTAL,)).astype(np.int64)
    got = kernel(xs, ts)
    m = np.where(np.arange(C)[None, :] == ts[:, None], xs, -xs)
    hinge = np.maximum(0.0, 1.0 - m)
    loss = np.where(m >= -1.0, hinge * hinge, -4.0 * m)
    want = loss.sum(dtype=np.float64) / N_TOTAL
    print("got", got, "want", want, "rel", abs(got - want) / abs(want))
